# revision 23
# baseline (speedup 1.0000x reference)
"""EMA recurrence kernel for Trainium2 (8 NeuronCores, Bass/Tile).

Computes a_t = w * x_t + (1 - w) * a_{t-1} over inputs [B=32, T=8192, C=128],
initial_state [B, C], weights [C] -> output [B, T, C].

Strategy: fp16 I/O (2x less HBM traffic) + even/odd phase decomposition so
the serial DVE scan only runs over HALF the time steps (the scan is
latency-bound at ~2.2 ns/column regardless of dtype). The other engines
carry everything else:

  a_{2k+1} = c^2 * a_{2k-1} + B_k,   B_k = (c*w)*x_{2k} + w*x_{2k+1}
  a_{2k}   = c * a_{2k-1} + w * x_{2k}

  - Batch dim sharded 4-per-core across 8 cores. Host marshals each core's
    slice to channel-major, even/odd phase-split fp16 ([C, T/2] per phase),
    so the device needs no transposes or strided accesses.
  - Per chunk of K=1024 scan columns (2048 time steps):
      * SP  : stream x_even / x_odd fp16 chunks HBM -> SBUF
      * PE  : B = diag(c*w) @ x_even + diag(w) @ x_odd       -> PSUM
      * DVE : fold c^2 * a_prev into B[:,0] (the scan initial is an
              IMMEDIATE 0.0 -- an AP initial costs ~1us/scan on HW),
              then a_odd = scan(c^2, B) reading B straight from PSUM
      * PE  : psE = diag(c) @ a_odd_shifted + diag(w) @ x_even -> PSUM
      * ACT : y_even = copy(psE) fp32 -> fp16
      * GPS : out-DMA y_even / a_odd chunks (SWDGE ring)
  - Per-channel one-shot factors (c*w, w, c) live in fp16 diag matrices;
    only the scan decay c^2 must stay fp32 (fp16 decay would cost ~1e-2
    accuracy for channels with c ~ 0.995).
  - a_odd tiles carry a leading seed column (s0 / previous chunk's last
    odd value), so the PE's "previous odd" operand is a 1-column-offset
    slice of the same tile.
  - The PE HAM clock gate starts throttled (~2.2x slower matmuls) and only
    releases after ~4us of sustained activity, so the kernel issues junk
    matmuls during the DMA fill phase to pre-warm it.
  - PE is software-pipelined: the next chunk's B matmuls are issued BEFORE
    the current chunk's psE matmuls, so the scan dependency never idles PE.
  - Host re-interleaves phases and converts back to fp32 [B, T, C].

Measured end-to-end max rel error ~8e-4 against the float64 reference.
"""

import sys

if "/opt/trn_rl_repo" not in sys.path:
    sys.path.insert(0, "/opt/trn_rl_repo")

import numpy as np

B, T, C = 32, 8192, 128
NCORES = 8
BL = B // NCORES      # batches per core (4)
TH = T // 2           # per-phase length (4096)
K = 1024              # scan columns per chunk
MM = 512              # matmul moving width (one PSUM bank)
NWARM = 10           # HAM pre-warming junk matmuls

_NC_CACHE = None


def build_bass():
    global _NC_CACHE
    if _NC_CACHE is not None:
        return _NC_CACHE

    import concourse.bacc as bacc
    import concourse.mybir as mybir
    import concourse.tile as tile

    f32 = mybir.dt.float32
    f16 = mybir.dt.float16
    AF = mybir.ActivationFunctionType
    ALU = mybir.AluOpType

    nc = bacc.Bacc("TRN2", target_bir_lowering=False, debug=False)
    xev = nc.dram_tensor("xev", [BL, C, TH], f16, kind="ExternalInput").ap()
    xod = nc.dram_tensor("xod", [BL, C, TH], f16, kind="ExternalInput").ap()
    s016 = nc.dram_tensor("s016", [C, BL], f16, kind="ExternalInput").ap()
    c2col = nc.dram_tensor("c2col", [C, 1], f32, kind="ExternalInput").ap()
    cdec2 = nc.dram_tensor("cdec2", [C, K], f32, kind="ExternalInput").ap()
    diagc = nc.dram_tensor("diagc", [C, C], f16, kind="ExternalInput").ap()
    diagw = nc.dram_tensor("diagw", [C, C], f16, kind="ExternalInput").ap()
    diagcw = nc.dram_tensor("diagcw", [C, C], f16, kind="ExternalInput").ap()
    yev = nc.dram_tensor("yev", [BL, C, TH], f16, kind="ExternalOutput").ap()
    yod = nc.dram_tensor("yod", [BL, C, TH], f16, kind="ExternalOutput").ap()

    with tile.TileContext(nc) as tc:
        with (
            tc.tile_pool(name="const", bufs=1) as cpool,
            tc.tile_pool(name="xin", bufs=10) as xpool,
            tc.tile_pool(name="out", bufs=4) as opool,
            tc.tile_pool(name="psb", bufs=2, space="PSUM") as bpool,
            tc.tile_pool(name="pse", bufs=2, space="PSUM") as epool,
        ):
            # consts on the idle GPS ring; x stream starts on SP immediately
            s016_t = cpool.tile([C, BL], f16, name="s016_t")
            nc.gpsimd.dma_start(s016_t[:], s016[:])
            c2col_t = cpool.tile([C, 1], f32, name="c2col_t")
            nc.gpsimd.dma_start(c2col_t[:], c2col[:])
            diagcw_t = cpool.tile([C, C], f16, name="diagcw_t")
            nc.gpsimd.dma_start(diagcw_t[:], diagcw[:])
            diagw_t = cpool.tile([C, C], f16, name="diagw_t")
            nc.gpsimd.dma_start(diagw_t[:], diagw[:])
            diagc_t = cpool.tile([C, C], f16, name="diagc_t")
            nc.gpsimd.dma_start(diagc_t[:], diagc[:])

            # scan decay c^2 [C, K] fp32, uploaded on the GPS ring
            cdec2_t = cpool.tile([C, K], f32, name="cdec2_t")
            nc.gpsimd.dma_start(cdec2_t[:], cdec2[:])

            # chunk work-list, round-robin across batches so consecutive
            # scans are independent (the fold's carry is 4 slots back)
            per_b = {
                0: [(0, 512), (512, 512)] + [(lo, K) for lo in range(K, TH, K)],
                1: [(lo, K) for lo in range(0, TH, K)],
                2: [(lo, K) for lo in range(0, TH, K)],
                3: [(lo, K) for lo in range(0, TH - K, K)]
                   + [(TH - K, 512), (TH - 512, 512)],
            }
            plan = []
            pos = [0] * BL
            while any(pos[b] < len(per_b[b]) for b in range(BL)):
                for b in range(BL):
                    if pos[b] < len(per_b[b]):
                        lo, kk = per_b[b][pos[b]]
                        plan.append((b, lo, kk))
                        pos[b] += 1
            N = len(plan)

            aodd = {}
            yevt = {}
            for b in range(BL):
                aodd[b] = opool.tile(
                    [C, TH + 1], f16, name=f"aodd{b}", tag="aodd"
                )
                yevt[b] = opool.tile([C, TH], f16, name=f"yev{b}", tag="yev")

            # HAM pre-warm: junk matmuls on a zeroed tile during DMA fill
            junk = cpool.tile([C, C], f16, name="junk")
            nc.vector.memset(junk[:], 0.0)
            jps = bpool.tile([C, K], f32, name="jps", tag="B")
            for _ in range(NWARM):
                nc.tensor.matmul(
                    jps[:, 0:C], junk[:], junk[:], start=True, stop=True
                )

            xe = {}
            xo = {}
            Bt = {}

            def dma_in(j):
                b, lo, kk = plan[j]
                xo[j] = xpool.tile([C, kk], f16, name=f"xo{j}", tag="xo")
                nc.sync.dma_start(xo[j][:], xod[b][:, lo : lo + kk])
                xe[j] = xpool.tile([C, kk], f16, name=f"xe{j}", tag="xe")
                nc.sync.dma_start(xe[j][:], xev[b][:, lo : lo + kk])

            def pe_pre(j):
                b, lo, kk = plan[j]
                Bt[j] = bpool.tile([C, kk], f32, name=f"B{j}", tag="B")
                for q in range(0, kk, MM):
                    s = slice(q, min(q + MM, kk))
                    nc.tensor.matmul(
                        Bt[j][:, s], diagcw_t[:], xe[j][:, s],
                        start=True, stop=False,
                    )
                    nc.tensor.matmul(
                        Bt[j][:, s], diagw_t[:], xo[j][:, s],
                        start=False, stop=True,
                    )

            # prologue: 4 chunks of input + first two B tiles in flight
            for _p in range(4):
                dma_in(_p)
            pe_pre(0)
            pe_pre(1)

            evac_q = []
            pend_od = {b: None for b in range(BL)}
            pend_ev = {b: None for b in range(BL)}

            def flush_evac(tail=False):
                b_, lo_, kk_, ps_ = evac_q.pop(0)
                nc.scalar.activation(
                    yevt[b_][:, lo_ : lo_ + kk_], ps_[:], AF.Copy
                )
                pend_ev[b_] = (
                    pend_ev[b_][0] if pend_ev[b_] else lo_,
                    lo_ + kk_,
                )
                thr = 1024 if b_ == BL - 1 else 2048
                if pend_ev[b_][1] - pend_ev[b_][0] >= thr or lo_ + kk_ == TH:
                    l0, l1 = pend_ev[b_]
                    eng = nc.scalar if tail else nc.gpsimd
                    eng.dma_start(
                        yev[b_][:, l0:l1], yevt[b_][:, l0:l1]
                    )
                    pend_ev[b_] = None

            for j in range(N):
                b, lo, kk = plan[j]
                if lo == 0:  # seed column: a_{-1} = s0
                    nc.vector.tensor_copy(
                        aodd[b][:, 0:1], s016_t[:, b : b + 1]
                    )
                if j + 4 < N:
                    dma_in(j + 4)
                if j + 2 < N:
                    pe_pre(j + 2)

                # fold c^2 * a_prev_odd into B[:,0]; scan with immediate 0
                nc.vector.scalar_tensor_tensor(
                    Bt[j][:, 0:1],
                    aodd[b][:, lo : lo + 1],
                    c2col_t[:],
                    Bt[j][:, 0:1],
                    op0=ALU.mult,
                    op1=ALU.add,
                )
                nc.vector.tensor_tensor_scan(
                    aodd[b][:, lo + 1 : lo + 1 + kk],
                    cdec2_t[:, 0:kk],
                    Bt[j][:],
                    0.0,
                    op0=ALU.mult,
                    op1=ALU.add,
                )
                pend_od[b] = (
                    pend_od[b][0] if pend_od[b] else lo,
                    lo + kk,
                )
                thr = 1024 if b == BL - 1 else 2048
                if pend_od[b][1] - pend_od[b][0] >= thr or lo + kk == TH:
                    l0, l1 = pend_od[b]
                    eng = nc.scalar if j >= N - 2 else nc.gpsimd
                    eng.dma_start(
                        yod[b][:, l0:l1], aodd[b][:, l0 + 1 : l1 + 1]
                    )
                    pend_od[b] = None

                # even phase: psE = diag(c) @ a_odd_shift + diag(w) @ x_even
                ps = epool.tile([C, kk], f32, name=f"ps{j}", tag="E")
                for q in range(0, kk, MM):
                    hi = min(q + MM, kk)
                    s = slice(q, hi)
                    nc.tensor.matmul(
                        ps[:, s], diagc_t[:],
                        aodd[b][:, lo + q : lo + hi],
                        start=True, stop=False,
                    )
                    nc.tensor.matmul(
                        ps[:, s], diagw_t[:], xe[j][:, s],
                        start=False, stop=True,
                    )

                evac_q.append((b, lo, kk, ps))
                if len(evac_q) > 1:
                    flush_evac(tail=(j >= N - 2))

            flush_evac(tail=True)

    nc.compile()
    _NC_CACHE = nc
    return nc


def _in_maps(inputs, initial_state, weights):
    x = np.asarray(inputs, dtype=np.float32)
    s0 = np.asarray(initial_state, dtype=np.float32)
    w = np.clip(np.asarray(weights, dtype=np.float32), 0.0, 1.0)
    c = (1.0 - w).astype(np.float32)

    c2col = np.ascontiguousarray((c.astype(np.float64) ** 2)[:, None]).astype(
        np.float32
    )
    cdec2m = np.ascontiguousarray(np.repeat(c2col, K, axis=1))
    diagc = np.diag(c).astype(np.float16)
    diagw = np.diag(w).astype(np.float16)
    diagcw = np.diag(c * w).astype(np.float16)

    maps = []
    for i in range(NCORES):
        xs = x[i * BL : (i + 1) * BL]  # [BL, T, C]
        xt = xs.transpose(0, 2, 1).astype(np.float16)  # [BL, C, T]
        maps.append(
            {
                "xev": np.ascontiguousarray(xt[:, :, 0::2]),
                "xod": np.ascontiguousarray(xt[:, :, 1::2]),
                "s016": np.ascontiguousarray(
                    s0[i * BL : (i + 1) * BL].T.astype(np.float16)
                ),
                "c2col": c2col,
                "cdec2": cdec2m,
                "diagc": diagc,
                "diagw": diagw,
                "diagcw": diagcw,
            }
        )
    return maps


def _gather(core_outs):
    """core_outs: list of (yev, yod) [BL, C, TH] fp16 -> [B, T, C] fp32."""
    out = np.empty((B, T, C), dtype=np.float32)
    y16 = np.empty((BL, C, T), dtype=np.float16)
    for i, (ye, yo) in enumerate(core_outs):
        y16[:, :, 0::2] = ye
        y16[:, :, 1::2] = yo
        out[i * BL : (i + 1) * BL] = y16.transpose(0, 2, 1).astype(np.float32)
    return out


def _ensure_ntff_hook():
    """Shim antenv.axon_hooks (absent in this image) so trace=True works."""
    import types

    import antenv

    if not hasattr(antenv, "axon_hooks"):
        mod = types.ModuleType("antenv.axon_hooks")
        holder = [None]
        mod.set_axon_ntff_profile_hook = lambda h: holder.__setitem__(0, h)
        mod.get_axon_ntff_profile_hook = lambda: holder[0]
        sys.modules["antenv.axon_hooks"] = mod
        antenv.axon_hooks = mod
    from antenv.axon_hooks import (
        get_axon_ntff_profile_hook,
        set_axon_ntff_profile_hook,
    )

    if get_axon_ntff_profile_hook() is None:
        from trn_agent_boot.trn_boot import _ntff_profile_via_ctypes

        set_axon_ntff_profile_hook(
            _ntff_profile_via_ctypes("/opt/axon/libaxon_pjrt.so")
        )


def run(inputs, initial_state, weights, trace=False, **kw):
    from concourse import bass_utils

    if trace:
        _ensure_ntff_hook()
    nc = build_bass()
    maps = _in_maps(inputs, initial_state, weights)
    res = bass_utils.run_bass_kernel_spmd(
        nc, maps, core_ids=list(range(NCORES)), trace=trace, **kw
    )
    out = _gather([(r["yev"], r["yod"]) for r in res.results])
    return out, res


def kernel(inputs, initial_state, weights):
    out, _ = run(inputs, initial_state, weights)
    return out


# revision 24
# speedup vs baseline: 1.0600x; 1.0600x over previous
"""EMA recurrence kernel for Trainium2 (8 NeuronCores, Bass/Tile).

Computes a_t = w * x_t + (1 - w) * a_{t-1} over inputs [B=32, T=8192, C=128],
initial_state [B, C], weights [C] -> output [B, T, C].

Strategy: fp16 I/O (2x less HBM traffic) + even/odd phase decomposition so
the serial DVE scan only runs over HALF the time steps (the scan is
latency-bound at ~2.2 ns/column regardless of dtype). The other engines
carry everything else:

  a_{2k+1} = c^2 * a_{2k-1} + B_k,   B_k = (c*w)*x_{2k} + w*x_{2k+1}
  a_{2k}   = c * a_{2k-1} + w * x_{2k}

  - Batch dim sharded 4-per-core across 8 cores. Host marshals each core's
    slice to channel-major, even/odd phase-split fp16 ([C, T/2] per phase),
    so the device needs no transposes or strided accesses.
  - Per chunk of K=1024 scan columns (2048 time steps):
      * SP  : stream x_even / x_odd fp16 chunks HBM -> SBUF
      * PE  : B = diag(c*w) @ x_even + diag(w) @ x_odd       -> PSUM
      * DVE : fold c^2 * a_prev into B[:,0] (the scan initial is an
              IMMEDIATE 0.0 -- an AP initial costs ~1us/scan on HW),
              then a_odd = scan(c^2, B) reading B straight from PSUM
      * PE  : psE = diag(c) @ a_odd_shifted + diag(w) @ x_even -> PSUM
      * ACT : y_even = copy(psE) fp32 -> fp16
      * GPS : out-DMA y_even / a_odd chunks (SWDGE ring)
  - Per-channel one-shot factors (c*w, w, c) live in fp16 diag matrices;
    only the scan decay c^2 must stay fp32 (fp16 decay would cost ~1e-2
    accuracy for channels with c ~ 0.995).
  - a_odd tiles carry a leading seed column (s0 / previous chunk's last
    odd value), so the PE's "previous odd" operand is a 1-column-offset
    slice of the same tile.
  - The PE HAM clock gate starts throttled (~2.2x slower matmuls) and only
    releases after ~4us of sustained activity, so the kernel issues junk
    matmuls during the DMA fill phase to pre-warm it.
  - PE is software-pipelined: the next chunk's B matmuls are issued BEFORE
    the current chunk's psE matmuls, so the scan dependency never idles PE.
  - Host re-interleaves phases and converts back to fp32 [B, T, C].

Measured end-to-end max rel error ~8e-4 against the float64 reference.
"""

import sys

if "/opt/trn_rl_repo" not in sys.path:
    sys.path.insert(0, "/opt/trn_rl_repo")

import numpy as np

B, T, C = 32, 8192, 128
NCORES = 8
BL = B // NCORES      # batches per core (4)
TH = T // 2           # per-phase length (4096)
K = 1024              # scan columns per chunk
MM = 512              # matmul moving width (one PSUM bank)
NWARM = 10           # HAM pre-warming junk matmuls

_NC_CACHE = None


def build_bass():
    global _NC_CACHE
    if _NC_CACHE is not None:
        return _NC_CACHE

    import concourse.bacc as bacc
    import concourse.mybir as mybir
    import concourse.tile as tile

    f32 = mybir.dt.float32
    f16 = mybir.dt.float16
    AF = mybir.ActivationFunctionType
    ALU = mybir.AluOpType

    nc = bacc.Bacc("TRN2", target_bir_lowering=False, debug=False)
    xev = nc.dram_tensor("xev", [BL, C, TH], f16, kind="ExternalInput").ap()
    xod = nc.dram_tensor("xod", [BL, C, TH], f16, kind="ExternalInput").ap()
    s016 = nc.dram_tensor("s016", [C, BL], f16, kind="ExternalInput").ap()
    c2col = nc.dram_tensor("c2col", [C, 1], f32, kind="ExternalInput").ap()
    diagc = nc.dram_tensor("diagc", [C, C], f16, kind="ExternalInput").ap()
    diagw = nc.dram_tensor("diagw", [C, C], f16, kind="ExternalInput").ap()
    diagcw = nc.dram_tensor("diagcw", [C, C], f16, kind="ExternalInput").ap()
    yev = nc.dram_tensor("yev", [BL, C, TH], f16, kind="ExternalOutput").ap()
    yod = nc.dram_tensor("yod", [BL, C, TH], f16, kind="ExternalOutput").ap()

    with tile.TileContext(nc) as tc:
        with (
            tc.tile_pool(name="const", bufs=1) as cpool,
            tc.tile_pool(name="xin", bufs=10) as xpool,
            tc.tile_pool(name="out", bufs=4) as opool,
            tc.tile_pool(name="psb", bufs=2, space="PSUM") as bpool,
            tc.tile_pool(name="pse", bufs=2, space="PSUM") as epool,
        ):
            # consts on the idle GPS ring; x stream starts on SP immediately
            s016_t = cpool.tile([C, BL], f16, name="s016_t")
            nc.gpsimd.dma_start(s016_t[:], s016[:])
            c2col_t = cpool.tile([C, 1], f32, name="c2col_t")
            nc.gpsimd.dma_start(c2col_t[:], c2col[:])
            diagcw_t = cpool.tile([C, C], f16, name="diagcw_t")
            nc.gpsimd.dma_start(diagcw_t[:], diagcw[:])
            diagw_t = cpool.tile([C, C], f16, name="diagw_t")
            nc.gpsimd.dma_start(diagw_t[:], diagw[:])
            diagc_t = cpool.tile([C, C], f16, name="diagc_t")
            nc.gpsimd.dma_start(diagc_t[:], diagc[:])

            # scan decay c^2 [C, K] fp32 built on device
            cdec2_t = cpool.tile([C, K], f32, name="cdec2_t")
            nc.vector.memset(cdec2_t[:], 1.0)
            nc.vector.tensor_scalar_mul(cdec2_t[:], cdec2_t[:], c2col_t[:])

            # chunk work-list, round-robin across batches so consecutive
            # scans are independent (the fold's carry is 4 slots back)
            per_b = {
                0: [(0, 512), (512, 512)] + [(lo, K) for lo in range(K, TH, K)],
                1: [(lo, K) for lo in range(0, TH, K)],
                2: [(lo, K) for lo in range(0, TH, K)],
                3: [(lo, K) for lo in range(0, TH - K, K)]
                   + [(TH - K, 512), (TH - 512, 512)],
            }
            plan = []
            pos = [0] * BL
            while any(pos[b] < len(per_b[b]) for b in range(BL)):
                for b in range(BL):
                    if pos[b] < len(per_b[b]):
                        lo, kk = per_b[b][pos[b]]
                        plan.append((b, lo, kk))
                        pos[b] += 1
            N = len(plan)

            aodd = {}
            yevt = {}
            for b in range(BL):
                aodd[b] = opool.tile(
                    [C, TH + 1], f16, name=f"aodd{b}", tag="aodd"
                )
                yevt[b] = opool.tile([C, TH], f16, name=f"yev{b}", tag="yev")

            # HAM pre-warm: junk matmuls on a zeroed tile during DMA fill
            junk = cpool.tile([C, C], f16, name="junk")
            nc.vector.memset(junk[:], 0.0)
            jps = bpool.tile([C, K], f32, name="jps", tag="B")
            for _ in range(NWARM):
                nc.tensor.matmul(
                    jps[:, 0:C], junk[:], junk[:], start=True, stop=True
                )

            xe = {}
            xo = {}
            Bt = {}

            def dma_in(j):
                b, lo, kk = plan[j]
                xo[j] = xpool.tile([C, kk], f16, name=f"xo{j}", tag="xo")
                nc.sync.dma_start(xo[j][:], xod[b][:, lo : lo + kk])
                xe[j] = xpool.tile([C, kk], f16, name=f"xe{j}", tag="xe")
                nc.sync.dma_start(xe[j][:], xev[b][:, lo : lo + kk])

            def pe_pre(j):
                b, lo, kk = plan[j]
                Bt[j] = bpool.tile([C, kk], f32, name=f"B{j}", tag="B")
                for q in range(0, kk, MM):
                    s = slice(q, min(q + MM, kk))
                    nc.tensor.matmul(
                        Bt[j][:, s], diagcw_t[:], xe[j][:, s],
                        start=True, stop=False,
                    )
                    nc.tensor.matmul(
                        Bt[j][:, s], diagw_t[:], xo[j][:, s],
                        start=False, stop=True,
                    )

            # prologue: 4 chunks of input + first two B tiles in flight
            for _p in range(4):
                dma_in(_p)
            pe_pre(0)
            pe_pre(1)

            evac_q = []
            pend_od = {b: None for b in range(BL)}
            pend_ev = {b: None for b in range(BL)}

            def flush_evac(tail=False):
                b_, lo_, kk_, ps_ = evac_q.pop(0)
                nc.scalar.activation(
                    yevt[b_][:, lo_ : lo_ + kk_], ps_[:], AF.Copy
                )
                pend_ev[b_] = (
                    pend_ev[b_][0] if pend_ev[b_] else lo_,
                    lo_ + kk_,
                )
                thr = 1024 if b_ == BL - 1 else 2048
                if pend_ev[b_][1] - pend_ev[b_][0] >= thr or lo_ + kk_ == TH:
                    l0, l1 = pend_ev[b_]
                    eng = nc.scalar if tail else nc.gpsimd
                    eng.dma_start(
                        yev[b_][:, l0:l1], yevt[b_][:, l0:l1]
                    )
                    pend_ev[b_] = None

            for j in range(N):
                b, lo, kk = plan[j]
                if lo == 0:  # seed column: a_{-1} = s0
                    nc.vector.tensor_copy(
                        aodd[b][:, 0:1], s016_t[:, b : b + 1]
                    )
                if j + 4 < N:
                    dma_in(j + 4)
                if j + 2 < N:
                    pe_pre(j + 2)

                # fold c^2 * a_prev_odd into B[:,0]; scan with immediate 0
                nc.vector.scalar_tensor_tensor(
                    Bt[j][:, 0:1],
                    aodd[b][:, lo : lo + 1],
                    c2col_t[:],
                    Bt[j][:, 0:1],
                    op0=ALU.mult,
                    op1=ALU.add,
                )
                nc.vector.tensor_tensor_scan(
                    aodd[b][:, lo + 1 : lo + 1 + kk],
                    cdec2_t[:, 0:kk],
                    Bt[j][:],
                    0.0,
                    op0=ALU.mult,
                    op1=ALU.add,
                )
                pend_od[b] = (
                    pend_od[b][0] if pend_od[b] else lo,
                    lo + kk,
                )
                thr = 1024 if b == BL - 1 else 2048
                if pend_od[b][1] - pend_od[b][0] >= thr or lo + kk == TH:
                    l0, l1 = pend_od[b]
                    eng = nc.scalar if j >= N - 2 else nc.gpsimd
                    eng.dma_start(
                        yod[b][:, l0:l1], aodd[b][:, l0 + 1 : l1 + 1]
                    )
                    pend_od[b] = None

                # even phase: psE = diag(c) @ a_odd_shift + diag(w) @ x_even
                ps = epool.tile([C, kk], f32, name=f"ps{j}", tag="E")
                for q in range(0, kk, MM):
                    hi = min(q + MM, kk)
                    s = slice(q, hi)
                    nc.tensor.matmul(
                        ps[:, s], diagc_t[:],
                        aodd[b][:, lo + q : lo + hi],
                        start=True, stop=False,
                    )
                    nc.tensor.matmul(
                        ps[:, s], diagw_t[:], xe[j][:, s],
                        start=False, stop=True,
                    )

                evac_q.append((b, lo, kk, ps))
                if len(evac_q) > 1:
                    flush_evac(tail=(j >= N - 2))

            flush_evac(tail=True)

    nc.compile()
    _NC_CACHE = nc
    return nc


def _in_maps(inputs, initial_state, weights):
    x = np.asarray(inputs, dtype=np.float32)
    s0 = np.asarray(initial_state, dtype=np.float32)
    w = np.clip(np.asarray(weights, dtype=np.float32), 0.0, 1.0)
    c = (1.0 - w).astype(np.float32)

    c2col = np.ascontiguousarray((c.astype(np.float64) ** 2)[:, None]).astype(
        np.float32
    )
    diagc = np.diag(c).astype(np.float16)
    diagw = np.diag(w).astype(np.float16)
    diagcw = np.diag(c * w).astype(np.float16)

    maps = []
    for i in range(NCORES):
        xs = x[i * BL : (i + 1) * BL]  # [BL, T, C]
        xt = xs.transpose(0, 2, 1).astype(np.float16)  # [BL, C, T]
        maps.append(
            {
                "xev": np.ascontiguousarray(xt[:, :, 0::2]),
                "xod": np.ascontiguousarray(xt[:, :, 1::2]),
                "s016": np.ascontiguousarray(
                    s0[i * BL : (i + 1) * BL].T.astype(np.float16)
                ),
                "c2col": c2col,
                "diagc": diagc,
                "diagw": diagw,
                "diagcw": diagcw,
            }
        )
    return maps


def _gather(core_outs):
    """core_outs: list of (yev, yod) [BL, C, TH] fp16 -> [B, T, C] fp32."""
    out = np.empty((B, T, C), dtype=np.float32)
    y16 = np.empty((BL, C, T), dtype=np.float16)
    for i, (ye, yo) in enumerate(core_outs):
        y16[:, :, 0::2] = ye
        y16[:, :, 1::2] = yo
        out[i * BL : (i + 1) * BL] = y16.transpose(0, 2, 1).astype(np.float32)
    return out


def _ensure_ntff_hook():
    """Shim antenv.axon_hooks (absent in this image) so trace=True works."""
    import types

    import antenv

    if not hasattr(antenv, "axon_hooks"):
        mod = types.ModuleType("antenv.axon_hooks")
        holder = [None]
        mod.set_axon_ntff_profile_hook = lambda h: holder.__setitem__(0, h)
        mod.get_axon_ntff_profile_hook = lambda: holder[0]
        sys.modules["antenv.axon_hooks"] = mod
        antenv.axon_hooks = mod
    from antenv.axon_hooks import (
        get_axon_ntff_profile_hook,
        set_axon_ntff_profile_hook,
    )

    if get_axon_ntff_profile_hook() is None:
        from trn_agent_boot.trn_boot import _ntff_profile_via_ctypes

        set_axon_ntff_profile_hook(
            _ntff_profile_via_ctypes("/opt/axon/libaxon_pjrt.so")
        )


def run(inputs, initial_state, weights, trace=False, **kw):
    from concourse import bass_utils

    if trace:
        _ensure_ntff_hook()
    nc = build_bass()
    maps = _in_maps(inputs, initial_state, weights)
    res = bass_utils.run_bass_kernel_spmd(
        nc, maps, core_ids=list(range(NCORES)), trace=trace, **kw
    )
    out = _gather([(r["yev"], r["yod"]) for r in res.results])
    return out, res


def kernel(inputs, initial_state, weights):
    out, _ = run(inputs, initial_state, weights)
    return out
